# revision 11
# baseline (speedup 1.0000x reference)
"""ECT transform kernel for Trainium2, SPMD over 8 NeuronCores.

Math (per sample b):
    nh[b,n,t] = sum_d x[b,n,d] * v[d,t]
    ect[b,r,t] = sum_n sigmoid(SCALE*(lin[r] - nh[b,n,t]))
    out[b] = ect[b] / max_{r,t} ect[b]

Sharding: data-parallel over batch (B=16 -> 2 samples per core).

Per-core program (partitions p = (b, t), b in {0,1}, t in 0..63):
  - PE: nh[(b,t), n] = w6.T @ x6 with K=6 = (d, b) indicator contraction,
    4 matmuls of N=512 into one PSUM tile (128, 2048).
  - ACT: for each r: sigmoid(-SCALE*nh + SCALE*lin[r]) over the whole PSUM
    tile with the fused per-partition accumulator (accum_out) producing
    ect[(b,t), r] directly.  The r axis is never materialized.
  - normalize: free-dim max (DVE) + per-half partition max (GPSIMD),
    reciprocal, per-partition scale.
  - DMA out with (b,t,r) -> (b,r,t) permute.
"""

import numpy as np

import concourse.bass as bass
import concourse.bacc as bacc
import concourse.bass_isa as bass_isa
import concourse.tile as tile
from concourse import mybir
from concourse.bass_utils import run_bass_kernel_spmd

B = 16
N = 2048
D = 3
T = 64
R = 64
RADIUS = 1.0
SCALE = 100.0
NCORES = 8
B_SH = B // NCORES  # 2 samples per core
P = B_SH * T        # 128 partitions = (b, t)

_LIN = np.linspace(-RADIUS, RADIUS, R, dtype=np.float32)


def build_bass():
    nc = bacc.Bacc("TRN2", target_bir_lowering=False, name="ect_transform")
    x6 = nc.dram_tensor("x6", (2 * D, N), mybir.dt.float32, kind="ExternalInput")
    w6 = nc.dram_tensor("w6", (2 * D, P), mybir.dt.float32, kind="ExternalInput")
    bt = nc.dram_tensor("bt", (P, R), mybir.dt.float32, kind="ExternalInput")
    out = nc.dram_tensor("out", (B_SH, R, T), mybir.dt.float32, kind="ExternalOutput")

    with (
        tile.TileContext(nc) as tc,
        tc.tile_pool(name="sb", bufs=1) as sb,
        tc.tile_pool(name="ps", bufs=1, space="PSUM") as ps,
    ):
        x6_sb = sb.tile([2 * D, N], mybir.dt.float32)
        w6_sb = sb.tile([2 * D, P], mybir.dt.float32)
        bt_sb = sb.tile([P, R], mybir.dt.float32)
        nc.sync.dma_start(out=x6_sb[:], in_=x6[:])
        nc.sync.dma_start(out=w6_sb[:], in_=w6[:])
        nc.sync.dma_start(out=bt_sb[:], in_=bt[:])

        nh_ps = ps.tile([P, N], mybir.dt.float32)   # 4 PSUM banks
        scr_ps = ps.tile([P, N], mybir.dt.float32)  # 4 PSUM banks (scratch)

        for j in range(N // 512):
            nc.tensor.matmul(
                nh_ps[:, 512 * j : 512 * (j + 1)],
                w6_sb[:],
                x6_sb[:, 512 * j : 512 * (j + 1)],
                start=True,
                stop=True,
            )

        ect = sb.tile([P, R], mybir.dt.float32)
        for r in range(R):
            nc.scalar.activation(
                scr_ps[:],
                nh_ps[:],
                mybir.ActivationFunctionType.Sigmoid,
                bias=bt_sb[:, r : r + 1],
                scale=-SCALE,
                accum_out=ect[:, r : r + 1],
            )

        # Per-sample normalization: max over (t, r) within each b half.
        m = sb.tile([P, 1], mybir.dt.float32)
        nc.vector.tensor_reduce(
            m[:], ect[:], axis=mybir.AxisListType.X, op=mybir.AluOpType.max
        )
        # HW gpsimd partition_all_reduce ignores a nonzero base partition, so
        # shift the b=1 half down to a base-0 tile, reduce both halves at
        # base 0, and shift back.
        mlow = sb.tile([T, 1], mybir.dt.float32)
        nc.sync.dma_start(out=mlow[:], in_=m[T:P, :])
        nc.gpsimd.partition_all_reduce(m[0:T], m[0:T], T, bass_isa.ReduceOp.max)
        nc.gpsimd.partition_all_reduce(mlow[:], mlow[:], T, bass_isa.ReduceOp.max)
        nc.sync.dma_start(out=m[T:P, :], in_=mlow[:])
        rec = sb.tile([P, 1], mybir.dt.float32)
        nc.vector.reciprocal(rec[:], m[:])
        ectn = sb.tile([P, R], mybir.dt.float32)
        nc.vector.tensor_scalar_mul(ectn[:], ect[:], rec[:])

        # (b,t) partitions x r free  ->  DRAM out[b, r, t]
        for b in range(B_SH):
            nc.sync.dma_start(
                out=out[b].rearrange("r t -> t r"),
                in_=ectn[b * T : (b + 1) * T, :],
            )

    nc.compile()
    return nc


def _make_w6(v):
    w6 = np.zeros((2 * D, P), dtype=np.float32)
    for d in range(D):
        for b in range(B_SH):
            w6[d * B_SH + b, b * T : (b + 1) * T] = v[d]
    return w6


def _make_bt():
    # bias table: column r = SCALE*lin[r], replicated across partitions
    return np.ascontiguousarray(
        np.tile((SCALE * _LIN)[None, :], (P, 1)).astype(np.float32)
    )


_NC_CACHE = {}


def _get_nc():
    if "nc" not in _NC_CACHE:
        _NC_CACHE["nc"] = build_bass()
    return _NC_CACHE["nc"]


def kernel(x, v, _trace=False):
    x = np.ascontiguousarray(np.asarray(x, dtype=np.float32))
    v = np.ascontiguousarray(np.asarray(v, dtype=np.float32))
    assert x.shape == (B, N, D) and v.shape == (D, T)

    w6 = _make_w6(v)
    bt = _make_bt()
    in_maps = []
    for c in range(NCORES):
        xs = x[B_SH * c : B_SH * (c + 1)]          # (2, 2048, 3)
        x6 = np.ascontiguousarray(
            xs.transpose(2, 0, 1).reshape(2 * D, N)  # rows (d, b) = d*2+b
        )
        in_maps.append({"x6": x6, "w6": w6, "bt": bt})

    nc = _get_nc()
    res = run_bass_kernel_spmd(
        nc, in_maps, core_ids=list(range(NCORES)), trace=_trace
    )
    out = np.concatenate([r["out"] for r in res.results], axis=0)
    if _trace:
        return out.astype(np.float32), res
    return out.astype(np.float32)


# revision 12
# speedup vs baseline: 1.1827x; 1.1827x over previous
"""ECT transform kernel for Trainium2, SPMD over 8 NeuronCores.

Math (per sample b):
    nh[b,n,t] = sum_d x[b,n,d] * v[d,t]
    ect[b,r,t] = sum_n sigmoid(SCALE*(lin[r] - nh[b,n,t]))
    out[b] = ect[b] / max_{r,t} ect[b]

Sharding: data-parallel over batch (B=16 -> 2 samples per core).

Per-core program (partitions p = (b, t), b in {0,1}, t in 0..63):
  - PE: nh[(b,t), n] = w18.T @ x18, K=18 = (split, d, b-indicator) with
    bf16-split precision (x_hi*v_hi + x_lo*v_hi + x_hi*v_lo), 4 matmuls of
    N=512 into one PSUM tile (128, 2048).  Single-pass bf16 (fp32 matmul
    runs as two passes on this PE).
  - ACT: for each r: sigmoid(-SCALE*nh + SCALE*lin[r]) over the whole PSUM
    tile, with the fused per-partition accumulator (accum_out) producing
    ect[(b,t), r] directly.  The r axis is never materialized.
  - normalize: free-dim max (DVE) + per-half partition max (GPSIMD at
    base partition 0), reciprocal, per-partition scale.
  - PE-transpose (identity matmul) to (r, (b,t)) and two contiguous
    output DMAs (strided 4-byte DMA to DRAM measured ~24us; transpose
    path is ~1us).
"""

import numpy as np
import ml_dtypes

import concourse.bass as bass
import concourse.bacc as bacc
import concourse.bass_isa as bass_isa
import concourse.tile as tile
from concourse import mybir
from concourse.bass_utils import run_bass_kernel_spmd
from concourse.masks import make_identity

B = 16
N = 2048
D = 3
T = 64
R = 64
RADIUS = 1.0
SCALE = 100.0
NCORES = 8
B_SH = B // NCORES  # 2 samples per core
P = B_SH * T        # 128 partitions = (b, t)
K = 18              # (3 precision terms) x (3 dims) x (2 b-indicator)

_LIN = np.linspace(-RADIUS, RADIUS, R, dtype=np.float32)
BF16 = ml_dtypes.bfloat16


def build_bass(scratch_sbuf=True):
    nc = bacc.Bacc("TRN2", target_bir_lowering=False, name="ect_transform")
    x18 = nc.dram_tensor("x18", (K, N), mybir.dt.bfloat16, kind="ExternalInput")
    w18 = nc.dram_tensor("w18", (K, P), mybir.dt.bfloat16, kind="ExternalInput")
    bt = nc.dram_tensor("bt", (P, R), mybir.dt.float32, kind="ExternalInput")
    out = nc.dram_tensor("out", (B_SH, R, T), mybir.dt.float32, kind="ExternalOutput")

    with (
        tile.TileContext(nc) as tc,
        tc.tile_pool(name="sb", bufs=1) as sb,
        tc.tile_pool(name="ps", bufs=1, space="PSUM") as ps,
    ):
        # Warm the sigmoid activation table concurrently with input DMAs.
        warm = sb.tile([P, 1], mybir.dt.float32)
        nc.vector.memset(warm[:], 0.0)
        nc.scalar.activation(
            warm[:], warm[:], mybir.ActivationFunctionType.Sigmoid, bias=warm[:]
        )

        x18_sb = sb.tile([K, N], mybir.dt.bfloat16)
        w18_sb = sb.tile([K, P], mybir.dt.bfloat16)
        bt_sb = sb.tile([P, R], mybir.dt.float32)
        nc.sync.dma_start(out=w18_sb[:], in_=w18[:])
        nc.sync.dma_start(out=bt_sb[:], in_=bt[:])

        nh_ps = ps.tile([P, N], mybir.dt.float32)  # 4 PSUM banks
        if scratch_sbuf:
            scr = sb.tile([P, N], mybir.dt.float32)
        else:
            scr = ps.tile([P, N], mybir.dt.float32)

        for j in range(N // 512):
            sl = slice(512 * j, 512 * (j + 1))
            nc.sync.dma_start(out=x18_sb[:, sl], in_=x18[:, sl])
            nc.tensor.matmul(
                nh_ps[:, sl], w18_sb[:], x18_sb[:, sl], start=True, stop=True
            )

        ect = sb.tile([P, R], mybir.dt.float32)
        for r in range(R):
            nc.scalar.activation(
                scr[:],
                nh_ps[:],
                mybir.ActivationFunctionType.Sigmoid,
                bias=bt_sb[:, r : r + 1],
                scale=-SCALE,
                accum_out=ect[:, r : r + 1],
            )

        # Per-sample normalization: max over (t, r) within each b half.
        m = sb.tile([P, 1], mybir.dt.float32)
        nc.vector.tensor_reduce(
            m[:], ect[:], axis=mybir.AxisListType.X, op=mybir.AluOpType.max
        )
        # HW gpsimd partition_all_reduce ignores a nonzero base partition, so
        # shift the b=1 half down to a base-0 tile, reduce both halves at
        # base 0, and shift back.
        mlow = sb.tile([T, 1], mybir.dt.float32)
        nc.sync.dma_start(out=mlow[:], in_=m[T:P, :])
        nc.gpsimd.partition_all_reduce(m[0:T], m[0:T], T, bass_isa.ReduceOp.max)
        nc.gpsimd.partition_all_reduce(mlow[:], mlow[:], T, bass_isa.ReduceOp.max)
        nc.sync.dma_start(out=m[T:P, :], in_=mlow[:])
        rec = sb.tile([P, 1], mybir.dt.float32)
        nc.vector.reciprocal(rec[:], m[:])
        ectn = sb.tile([P, R], mybir.dt.float32)
        nc.vector.tensor_scalar_mul(ectn[:], ect[:], rec[:])

        # Transpose (b,t) x r -> r x (b,t) on the PE, then contiguous DMAs.
        ident = sb.tile([P, P], mybir.dt.float32)
        make_identity(nc, ident[:])
        tp_ps = ps.tile([R, P], mybir.dt.float32)
        nc.tensor.transpose(tp_ps[:], ectn[:], ident[:])
        out_sb = sb.tile([R, P], mybir.dt.float32)
        nc.vector.tensor_copy(out_sb[:], tp_ps[:])
        for b in range(B_SH):
            nc.sync.dma_start(
                out=out[b], in_=out_sb[:, b * T : (b + 1) * T]
            )

    nc.compile()
    return nc


def _make_w18_x18(v, xs):
    """xs: (2, N, D) f32 shard.  Returns (w18 (K,P) bf16, x18 (K,N) bf16).

    Row k = s*6 + d*2 + kb encodes precision term s, dim d, sample kb:
      s=0: x_hi * v_hi ; s=1: x_lo * v_hi ; s=2: x_hi * v_lo
    """
    v_hi = v.astype(BF16)
    v_lo = (v - v_hi.astype(np.float32)).astype(BF16)
    x_hi = xs.astype(BF16)
    x_lo = (xs - x_hi.astype(np.float32)).astype(BF16)
    w18 = np.zeros((K, P), dtype=BF16)
    x18 = np.zeros((K, N), dtype=BF16)
    for s, (vv, xx) in enumerate(((v_hi, x_hi), (v_hi, x_lo), (v_lo, x_hi))):
        for d in range(D):
            for kb in range(B_SH):
                w18[s * 6 + d * 2 + kb, kb * T : (kb + 1) * T] = vv[d]
                x18[s * 6 + d * 2 + kb, :] = xx[kb, :, d]
    return w18, x18


def _make_bt():
    # bias table: column r = SCALE*lin[r], replicated across partitions
    return np.ascontiguousarray(
        np.tile((SCALE * _LIN)[None, :], (P, 1)).astype(np.float32)
    )


_NC_CACHE = {}


def _get_nc():
    if "nc" not in _NC_CACHE:
        _NC_CACHE["nc"] = build_bass()
    return _NC_CACHE["nc"]


def kernel(x, v, _trace=False, _nc=None):
    x = np.ascontiguousarray(np.asarray(x, dtype=np.float32))
    v = np.ascontiguousarray(np.asarray(v, dtype=np.float32))
    assert x.shape == (B, N, D) and v.shape == (D, T)

    bt = _make_bt()
    in_maps = []
    for c in range(NCORES):
        w18, x18 = _make_w18_x18(v, x[B_SH * c : B_SH * (c + 1)])
        in_maps.append({"x18": x18, "w18": w18, "bt": bt})

    nc = _nc if _nc is not None else _get_nc()
    res = run_bass_kernel_spmd(
        nc, in_maps, core_ids=list(range(NCORES)), trace=_trace
    )
    out = np.concatenate([r["out"] for r in res.results], axis=0)
    if _trace:
        return out.astype(np.float32), res
    return out.astype(np.float32)


# revision 14
# speedup vs baseline: 1.2059x; 1.0196x over previous
"""ECT transform kernel for Trainium2, SPMD over 8 NeuronCores.

Math (per sample b):
    nh[b,n,t] = sum_d x[b,n,d] * v[d,t]
    ect[b,r,t] = sum_n sigmoid(SCALE*(lin[r] - nh[b,n,t]))
    out[b] = ect[b] / max_{r,t} ect[b]

Sharding: data-parallel over batch (B=16 -> 2 samples per core).

Per-core program (partitions p = (b, t), b in {0,1}, t in 0..63):
  - PE: nh[(b,t), n] = w18.T @ x18, K=18 = (split, d, b-indicator) with
    bf16-split precision (x_hi*v_hi + x_lo*v_hi + x_hi*v_lo), 4 matmuls of
    N=512 into one PSUM tile (128, 2048).  Single-pass bf16 (fp32 matmul
    runs as two passes on this PE).
  - ACT: for each r: sigmoid(-SCALE*nh + SCALE*lin[r]) over the whole PSUM
    tile, with the fused per-partition accumulator (accum_out) producing
    ect[(b,t), r] directly.  The r axis is never materialized.
  - normalize: free-dim max (DVE) + per-half partition max (GPSIMD at
    base partition 0), reciprocal, per-partition scale.
  - PE-transpose (identity matmul) to (r, (b,t)) and two contiguous
    output DMAs (strided 4-byte DMA to DRAM measured ~24us; transpose
    path is ~1us).
"""

import numpy as np
import ml_dtypes

import concourse.bass as bass
import concourse.bacc as bacc
import concourse.bass_isa as bass_isa
import concourse.tile as tile
from concourse import mybir
from concourse.bass_utils import run_bass_kernel_spmd
from concourse.masks import make_identity

B = 16
N = 2048
D = 3
T = 64
R = 64
RADIUS = 1.0
SCALE = 100.0
NCORES = 8
B_SH = B // NCORES  # 2 samples per core
P = B_SH * T        # 128 partitions = (b, t)
K = 18              # (3 precision terms) x (3 dims) x (2 b-indicator)

_LIN = np.linspace(-RADIUS, RADIUS, R, dtype=np.float32)
BF16 = ml_dtypes.bfloat16


def build_bass(scratch_sbuf=True):
    nc = bacc.Bacc("TRN2", target_bir_lowering=False, name="ect_transform")
    x18 = nc.dram_tensor("x18", (K, N), mybir.dt.bfloat16, kind="ExternalInput")
    w18 = nc.dram_tensor("w18", (K, P), mybir.dt.bfloat16, kind="ExternalInput")
    bt = nc.dram_tensor("bt", (P, R), mybir.dt.float32, kind="ExternalInput")
    out = nc.dram_tensor("out", (B_SH, R, T), mybir.dt.float32, kind="ExternalOutput")

    with (
        tile.TileContext(nc) as tc,
        tc.tile_pool(name="sb", bufs=1) as sb,
        tc.tile_pool(name="ps", bufs=1, space="PSUM") as ps,
    ):
        # Warm the sigmoid activation table concurrently with input DMAs.
        warm = sb.tile([P, 1], mybir.dt.float32)
        nc.vector.memset(warm[:], 0.0)
        nc.scalar.activation(
            warm[:], warm[:], mybir.ActivationFunctionType.Sigmoid, bias=warm[:]
        )

        x18_sb = sb.tile([K, N], mybir.dt.bfloat16)
        w18_sb = sb.tile([K, P], mybir.dt.bfloat16)
        bt_sb = sb.tile([P, R], mybir.dt.float32)
        nc.sync.dma_start(out=x18_sb[:], in_=x18[:])
        nc.sync.dma_start(out=w18_sb[:], in_=w18[:])
        nc.sync.dma_start(out=bt_sb[:], in_=bt[:])

        ident = sb.tile([P, P], mybir.dt.float32)
        make_identity(nc, ident[:])
        ones = sb.tile([1, P], mybir.dt.float32)
        nc.vector.memset(ones[:], 1.0)

        nh_ps = ps.tile([P, N], mybir.dt.float32)  # 4 PSUM banks
        if scratch_sbuf:
            scr = sb.tile([P, N], mybir.dt.float32)
        else:
            scr = ps.tile([P, N], mybir.dt.float32)

        for j in range(N // 512):
            sl = slice(512 * j, 512 * (j + 1))
            nc.tensor.matmul(
                nh_ps[:, sl], w18_sb[:], x18_sb[:, sl], start=True, stop=True
            )

        ect = sb.tile([P, R], mybir.dt.float32)
        for r in range(R):
            nc.scalar.activation(
                scr[:],
                nh_ps[:],
                mybir.ActivationFunctionType.Sigmoid,
                bias=bt_sb[:, r : r + 1],
                scale=-SCALE,
                accum_out=ect[:, r : r + 1],
            )

        # Per-sample normalization: max over (t, r) within each b half.
        # All cross-partition movement happens on the PE (transpose +
        # K=1 broadcast matmul) -- no gpsimd, no DMA round-trips.
        m = sb.tile([P, 1], mybir.dt.float32)
        nc.vector.tensor_reduce(
            m[:], ect[:], axis=mybir.AxisListType.X, op=mybir.AluOpType.max
        )
        mT_ps = ps.tile([1, P], mybir.dt.float32)
        nc.tensor.transpose(mT_ps[:], m[:], ident[:])
        mrow = sb.tile([1, P], mybir.dt.float32)
        nc.vector.tensor_copy(mrow[:], mT_ps[:])
        m2 = sb.tile([1, B_SH], mybir.dt.float32)
        nc.vector.tensor_reduce(
            m2[:],
            mrow.rearrange("p (b t) -> p b t", b=B_SH),
            axis=mybir.AxisListType.X,
            op=mybir.AluOpType.max,
        )
        rec2 = sb.tile([1, B_SH], mybir.dt.float32)
        nc.vector.reciprocal(rec2[:], m2[:])
        recb_ps = ps.tile([P, B_SH], mybir.dt.float32)
        nc.tensor.matmul(recb_ps[:], ones[:], rec2[:], start=True, stop=True)
        recb = sb.tile([P, B_SH], mybir.dt.float32)
        nc.vector.tensor_copy(recb[:], recb_ps[:])
        ectn = sb.tile([P, R], mybir.dt.float32)
        for b in range(B_SH):
            nc.vector.tensor_scalar_mul(
                ectn[b * T : (b + 1) * T, :],
                ect[b * T : (b + 1) * T, :],
                recb[b * T : (b + 1) * T, b : b + 1],
            )

        # Transpose (b,t) x r -> r x (b,t) on the PE, then contiguous DMAs.
        tp_ps = ps.tile([R, P], mybir.dt.float32)
        nc.tensor.transpose(tp_ps[:], ectn[:], ident[:])
        out_sb = sb.tile([R, P], mybir.dt.float32)
        nc.vector.tensor_copy(out_sb[:], tp_ps[:])
        for b in range(B_SH):
            nc.sync.dma_start(
                out=out[b], in_=out_sb[:, b * T : (b + 1) * T]
            )

    nc.compile()
    return nc


def _make_w18_x18(v, xs):
    """xs: (2, N, D) f32 shard.  Returns (w18 (K,P) bf16, x18 (K,N) bf16).

    Row k = s*6 + d*2 + kb encodes precision term s, dim d, sample kb:
      s=0: x_hi * v_hi ; s=1: x_lo * v_hi ; s=2: x_hi * v_lo
    """
    v_hi = v.astype(BF16)
    v_lo = (v - v_hi.astype(np.float32)).astype(BF16)
    x_hi = xs.astype(BF16)
    x_lo = (xs - x_hi.astype(np.float32)).astype(BF16)
    w18 = np.zeros((K, P), dtype=BF16)
    x18 = np.zeros((K, N), dtype=BF16)
    for s, (vv, xx) in enumerate(((v_hi, x_hi), (v_hi, x_lo), (v_lo, x_hi))):
        for d in range(D):
            for kb in range(B_SH):
                w18[s * 6 + d * 2 + kb, kb * T : (kb + 1) * T] = vv[d]
                x18[s * 6 + d * 2 + kb, :] = xx[kb, :, d]
    return w18, x18


def _make_bt():
    # bias table: column r = SCALE*lin[r], replicated across partitions
    return np.ascontiguousarray(
        np.tile((SCALE * _LIN)[None, :], (P, 1)).astype(np.float32)
    )


_NC_CACHE = {}


def _get_nc():
    if "nc" not in _NC_CACHE:
        _NC_CACHE["nc"] = build_bass()
    return _NC_CACHE["nc"]


def kernel(x, v, _trace=False, _nc=None):
    x = np.ascontiguousarray(np.asarray(x, dtype=np.float32))
    v = np.ascontiguousarray(np.asarray(v, dtype=np.float32))
    assert x.shape == (B, N, D) and v.shape == (D, T)

    bt = _make_bt()
    in_maps = []
    for c in range(NCORES):
        w18, x18 = _make_w18_x18(v, x[B_SH * c : B_SH * (c + 1)])
        in_maps.append({"x18": x18, "w18": w18, "bt": bt})

    nc = _nc if _nc is not None else _get_nc()
    res = run_bass_kernel_spmd(
        nc, in_maps, core_ids=list(range(NCORES)), trace=_trace
    )
    out = np.concatenate([r["out"] for r in res.results], axis=0)
    if _trace:
        return out.astype(np.float32), res
    return out.astype(np.float32)


# revision 15
# speedup vs baseline: 1.2069x; 1.0008x over previous
"""ECT transform kernel for Trainium2, SPMD over 8 NeuronCores.

Math (per sample b):
    nh[b,n,t] = sum_d x[b,n,d] * v[d,t]
    ect[b,r,t] = sum_n sigmoid(SCALE*(lin[r] - nh[b,n,t]))
    out[b] = ect[b] / max_{r,t} ect[b]

Sharding: data-parallel over batch (B=16 -> 2 samples per core).

Per-core program (partitions p = (b, t), b in {0,1}, t in 0..63):
  - PE: nh[(b,t), n] = w18.T @ x18, K=18 = (split, d, b-indicator) with
    bf16-split precision (x_hi*v_hi + x_lo*v_hi + x_hi*v_lo), 4 matmuls of
    N=512 into one PSUM tile (128, 2048).  Single-pass bf16 (fp32 matmul
    runs as two passes on this PE).
  - ACT: for each r: sigmoid(-SCALE*nh + SCALE*lin[r]) over the whole PSUM
    tile, with the fused per-partition accumulator (accum_out) producing
    ect[(b,t), r] directly.  The r axis is never materialized.
  - normalize: free-dim max (DVE) + per-half partition max (GPSIMD at
    base partition 0), reciprocal, per-partition scale.
  - PE-transpose (identity matmul) to (r, (b,t)) and two contiguous
    output DMAs (strided 4-byte DMA to DRAM measured ~24us; transpose
    path is ~1us).
"""

import numpy as np
import ml_dtypes

import concourse.bass as bass
import concourse.bacc as bacc
import concourse.bass_isa as bass_isa
import concourse.tile as tile
from concourse import mybir
from concourse.bass_utils import run_bass_kernel_spmd
from concourse.masks import make_identity

B = 16
N = 2048
D = 3
T = 64
R = 64
RADIUS = 1.0
SCALE = 100.0
NCORES = 8
B_SH = B // NCORES  # 2 samples per core
P = B_SH * T        # 128 partitions = (b, t)
K = 18              # (3 precision terms) x (3 dims) x (2 b-indicator)

_LIN = np.linspace(-RADIUS, RADIUS, R, dtype=np.float32)
BF16 = ml_dtypes.bfloat16


def build_bass(scratch_sbuf=True):
    nc = bacc.Bacc("TRN2", target_bir_lowering=False, name="ect_transform")
    x18 = nc.dram_tensor("x18", (K, N), mybir.dt.bfloat16, kind="ExternalInput")
    w18 = nc.dram_tensor("w18", (K, P), mybir.dt.bfloat16, kind="ExternalInput")
    bt = nc.dram_tensor("bt", (P, R), mybir.dt.float32, kind="ExternalInput")
    out = nc.dram_tensor("out", (B_SH, R, T), mybir.dt.float32, kind="ExternalOutput")

    with (
        tile.TileContext(nc) as tc,
        tc.tile_pool(name="sb", bufs=1) as sb,
        tc.tile_pool(name="ps", bufs=1, space="PSUM") as ps,
    ):
        # Warm the sigmoid activation table concurrently with input DMAs.
        warm = sb.tile([P, 1], mybir.dt.float32)
        nc.vector.memset(warm[:], 0.0)
        nc.scalar.activation(
            warm[:], warm[:], mybir.ActivationFunctionType.Sigmoid, bias=warm[:]
        )

        x18_sb = sb.tile([K, N], mybir.dt.bfloat16)
        w18_sb = sb.tile([K, P], mybir.dt.bfloat16)
        bt_sb = sb.tile([P, R], mybir.dt.float32)
        nc.sync.dma_start(out=x18_sb[:], in_=x18[:])
        nc.sync.dma_start(out=w18_sb[:], in_=w18[:])
        nc.sync.dma_start(out=bt_sb[:], in_=bt[:])

        ident = sb.tile([P, P], mybir.dt.float32)
        make_identity(nc, ident[:])
        ones = sb.tile([1, P], mybir.dt.float32)
        nc.vector.memset(ones[:], 1.0)

        nh_ps = ps.tile([P, N], mybir.dt.float32)  # 4 PSUM banks
        if scratch_sbuf:
            # bf16 main output: the sigmoid values we keep come from the
            # f32 accumulator; the elementwise output is scratch.
            scr = sb.tile([P, N], mybir.dt.bfloat16)
        else:
            scr = ps.tile([P, N], mybir.dt.float32)

        for j in range(N // 512):
            sl = slice(512 * j, 512 * (j + 1))
            nc.tensor.matmul(
                nh_ps[:, sl], w18_sb[:], x18_sb[:, sl], start=True, stop=True
            )

        ect = sb.tile([P, R], mybir.dt.float32)
        for r in range(R):
            nc.scalar.activation(
                scr[:],
                nh_ps[:],
                mybir.ActivationFunctionType.Sigmoid,
                bias=bt_sb[:, r : r + 1],
                scale=-SCALE,
                accum_out=ect[:, r : r + 1],
            )

        # Per-sample normalization: max over (t, r) within each b half.
        # All cross-partition movement happens on the PE (transpose +
        # K=1 broadcast matmul) -- no gpsimd, no DMA round-trips.
        m = sb.tile([P, 1], mybir.dt.float32)
        nc.vector.tensor_reduce(
            m[:], ect[:], axis=mybir.AxisListType.X, op=mybir.AluOpType.max
        )
        mT_ps = ps.tile([1, P], mybir.dt.float32)
        nc.tensor.transpose(mT_ps[:], m[:], ident[:])
        mrow = sb.tile([1, P], mybir.dt.float32)
        nc.vector.tensor_copy(mrow[:], mT_ps[:])
        m2 = sb.tile([1, B_SH], mybir.dt.float32)
        nc.vector.tensor_reduce(
            m2[:],
            mrow.rearrange("p (b t) -> p b t", b=B_SH),
            axis=mybir.AxisListType.X,
            op=mybir.AluOpType.max,
        )
        rec2 = sb.tile([1, B_SH], mybir.dt.float32)
        nc.vector.reciprocal(rec2[:], m2[:])
        recb_ps = ps.tile([P, B_SH], mybir.dt.float32)
        nc.tensor.matmul(recb_ps[:], ones[:], rec2[:], start=True, stop=True)
        recb = sb.tile([P, B_SH], mybir.dt.float32)
        nc.vector.tensor_copy(recb[:], recb_ps[:])
        ectn = sb.tile([P, R], mybir.dt.float32)
        for b in range(B_SH):
            nc.vector.tensor_scalar_mul(
                ectn[b * T : (b + 1) * T, :],
                ect[b * T : (b + 1) * T, :],
                recb[b * T : (b + 1) * T, b : b + 1],
            )

        # Transpose (b,t) x r -> r x (b,t) on the PE, then contiguous DMAs.
        tp_ps = ps.tile([R, P], mybir.dt.float32)
        nc.tensor.transpose(tp_ps[:], ectn[:], ident[:])
        out_sb = sb.tile([R, P], mybir.dt.float32)
        nc.vector.tensor_copy(out_sb[:], tp_ps[:])
        for b in range(B_SH):
            nc.sync.dma_start(
                out=out[b], in_=out_sb[:, b * T : (b + 1) * T]
            )

    nc.compile()
    return nc


def _make_w18_x18(v, xs):
    """xs: (2, N, D) f32 shard.  Returns (w18 (K,P) bf16, x18 (K,N) bf16).

    Row k = s*6 + d*2 + kb encodes precision term s, dim d, sample kb:
      s=0: x_hi * v_hi ; s=1: x_lo * v_hi ; s=2: x_hi * v_lo
    """
    v_hi = v.astype(BF16)
    v_lo = (v - v_hi.astype(np.float32)).astype(BF16)
    x_hi = xs.astype(BF16)
    x_lo = (xs - x_hi.astype(np.float32)).astype(BF16)
    w18 = np.zeros((K, P), dtype=BF16)
    x18 = np.zeros((K, N), dtype=BF16)
    for s, (vv, xx) in enumerate(((v_hi, x_hi), (v_hi, x_lo), (v_lo, x_hi))):
        for d in range(D):
            for kb in range(B_SH):
                w18[s * 6 + d * 2 + kb, kb * T : (kb + 1) * T] = vv[d]
                x18[s * 6 + d * 2 + kb, :] = xx[kb, :, d]
    return w18, x18


def _make_bt():
    # bias table: column r = SCALE*lin[r], replicated across partitions
    return np.ascontiguousarray(
        np.tile((SCALE * _LIN)[None, :], (P, 1)).astype(np.float32)
    )


_NC_CACHE = {}


def _get_nc():
    if "nc" not in _NC_CACHE:
        _NC_CACHE["nc"] = build_bass()
    return _NC_CACHE["nc"]


def kernel(x, v, _trace=False, _nc=None):
    x = np.ascontiguousarray(np.asarray(x, dtype=np.float32))
    v = np.ascontiguousarray(np.asarray(v, dtype=np.float32))
    assert x.shape == (B, N, D) and v.shape == (D, T)

    bt = _make_bt()
    in_maps = []
    for c in range(NCORES):
        w18, x18 = _make_w18_x18(v, x[B_SH * c : B_SH * (c + 1)])
        in_maps.append({"x18": x18, "w18": w18, "bt": bt})

    nc = _nc if _nc is not None else _get_nc()
    res = run_bass_kernel_spmd(
        nc, in_maps, core_ids=list(range(NCORES)), trace=_trace
    )
    out = np.concatenate([r["out"] for r in res.results], axis=0)
    if _trace:
        return out.astype(np.float32), res
    return out.astype(np.float32)
